# revision 1
# baseline (speedup 1.0000x reference)
"""Trainium2 Bass kernel for CompanySpecificHeads (MoE-style routed MLP heads).

Semantics (matching the reference):
    out[b] = gelu(z[b] @ W1[cid[b]] + b1[cid[b]]) @ W2[cid[b]] + b2[cid[b]]

Strategy: expert-parallel across 8 NeuronCores. Companies are sharded
8-per-core; tokens are routed (gathered by company) to their company's core
on the host, padded to a fixed per-company capacity, and each core runs a
grouped GEMM -> gelu -> dot pipeline over its 8 companies:

  Layer 1 (per company c, h on partitions):
      psum[h, t] = sum_d W1[c][d, h] * zT[c][d, t]      (PE, fp16 operands)
      bias b1 is folded in with a K=4 "selector" matmul that broadcasts
      b1[c][128k+m] across the token axis before accumulation.
  Gelu: ACT engine, PSUM -> SBUF (fp16 out).
  Layer 2: psum2[1, t] += W2[c][hj]^T @ gelu_h[hj, t]   (8 K=128 matmuls)

Host does the unshard/scatter back to [B, 1] and adds b2 (exact, fp32).

DMA discipline: the DIRECT2D DMA encoding supports a single sync wait, so
the kernel keeps every DMACopy at <=1 wait: all loads target fresh SBUF
slots (no reuse -> no release waits), there are <=8 DMAs per DGE flavor
(fresh lane -> no FIFO wait), and the 8 per-company outputs are staged into
one persistent SBUF tile and stored with a single DMA at the end.
"""

import numpy as np

B, C, D, H = 4096, 64, 512, 1024
NCORES = 8
CPC = C // NCORES  # companies per core
KC = D // 128      # contraction chunks of 128
HC = H // 128      # h chunks of 128

_COMPILED = {}


def _build(TW, NTT, dtype_name):
    """Build the Bass/Tile program for per-company token capacity NTT*TW."""
    import concourse.bass as bass
    import concourse.bacc as bacc
    import concourse.mybir as mybir
    from concourse.tile import TileContext
    from contextlib import ExitStack

    f32 = mybir.dt.float32
    dt_op = getattr(mybir.dt, dtype_name)

    SELW = KC * TW           # selector columns
    B1W = CPC * 2 * 128      # b1 columns

    nc = bacc.Bacc(None, target_bir_lowering=False)

    # zt is stored partition-major so one DMA moves it with large packets.
    zt_d = nc.dram_tensor("zt", [128, CPC, NTT, KC, TW], dt_op, kind="ExternalInput")
    # w1 stored as [c][p][g][k][h-half]: a whole company loads linearly
    # with 8KB contiguous per partition (full-rate packets).
    w1_d = nc.dram_tensor(
        "w1", [CPC, 128, 2, KC, H // 2], dt_op, kind="ExternalInput"
    )
    cst_d = nc.dram_tensor("cst", [KC, SELW + B1W], dt_op, kind="ExternalInput")
    w2_d = nc.dram_tensor("w2h", [128, CPC * HC], dt_op, kind="ExternalInput")
    out_d = nc.dram_tensor("out", [1, CPC * NTT * TW], f32, kind="ExternalOutput")

    gelu = mybir.ActivationFunctionType.Gelu

    with TileContext(nc) as tc, ExitStack() as ctx:
        const = ctx.enter_context(tc.tile_pool(name="const", bufs=1))
        # Small constants: [sel | b1h] (4 partitions) and w2 (128 partitions).
        # On the ACT HWDGE ring so they land before w1[0] and the PE can
        # run the bias matmuls while weights stream in.
        ct = const.tile([KC, SELW + B1W], dt_op)
        nc.gpsimd.dma_start(out=ct[:], in_=cst_d[:])
        selt = ct[:, 0:SELW]
        b1t = ct[:, SELW:SELW + B1W].rearrange("p (c g m) -> p c g m", c=CPC, g=2)
        w2t = const.tile([128, CPC * HC], dt_op)
        nc.gpsimd.dma_start(out=w2t[:], in_=w2_d[:])

        # Routed tokens on the ACT HWDGE ring (its dispatch overlaps the SP
        # ring's w1 dispatches): first two companies land early so the PE
        # can start as soon as w1[0] arrives.
        zall = const.tile([128, CPC, NTT, KC, TW], dt_op)
        zsplit = 1
        nc.scalar.dma_start(out=zall[:, :zsplit], in_=zt_d[:, :zsplit])
        if zsplit < CPC:
            nc.scalar.dma_start(out=zall[:, zsplit:], in_=zt_d[:, zsplit:])

        # Staged per-company outputs; single store at the end (SWDGE).
        oall = const.tile([1, CPC * NTT * TW], f32)

        # Per-company weights on the SP HWDGE ring, half a company per DMA.
        # The ring drains FIFO at full bandwidth, so w1 chunks complete in
        # issue order and compute pipelines behind the weight stream.
        w1p = ctx.enter_context(tc.tile_pool(name="w1p", bufs=1))
        w1ts = []
        for c in range(CPC):
            w1t = w1p.tile([128, 2, KC, H // 2], dt_op, name=f"w1_{c}")
            # One DMA per company: 8KB contiguous per partition keeps the
            # SP ring at full rate (~343 GB/s measured).
            nc.sync.dma_start(out=w1t[:], in_=w1_d[c])
            w1ts.append(w1t)

        hp = ctx.enter_context(tc.tile_pool(name="hp", bufs=min(2 * CPC * NTT, 16)))
        pp = ctx.enter_context(tc.tile_pool(name="pp", bufs=5, space="PSUM"))
        opp = ctx.enter_context(tc.tile_pool(name="opp", bufs=2, space="PSUM"))

        # PE warmup: the PE sits idle ~7us while weights stream in, which
        # drops its clock to 1.2GHz (HAM cold state) right when real work
        # starts. Keep it busy with dependency-free matmuls on scratch data
        # so the first real matmul runs at the warm 2.4GHz clock.
        wsc = const.tile([128, 512], dt_op)
        nc.gpsimd.memset(wsc[:], 0.0)
        wps = ctx.enter_context(tc.tile_pool(name="wps", bufs=1, space="PSUM"))
        wp = wps.tile([128, 512], f32)
        for _ in range(22):
            nc.tensor.matmul(wp[:], wsc[:, :128], wsc[:], start=True, stop=True)

        for c in range(CPC):
            w1t = w1ts[c]
            for tt in range(NTT):
                osum = opp.tile([1, TW], f32)
                for g in range(2):
                    ps = pp.tile([128, KC * TW], f32)
                    # bias: ps[128k+m, (j,t)] = b1[c][512g+128j+m] via selector
                    nc.tensor.matmul(
                        ps[:], b1t[:, c, g, :], selt[:], start=True, stop=False
                    )
                    for j in range(KC):
                        jj = KC * g + j
                        for k in range(KC):
                            nc.tensor.matmul(
                                ps[:, j * TW:(j + 1) * TW],
                                w1t[:, g, k, 128 * j:128 * (j + 1)],
                                zall[:, c, tt, k, :],
                                start=False,
                                stop=(k == KC - 1),
                            )
                    ht = hp.tile([128, KC * TW], dt_op)
                    nc.scalar.activation(ht[:], ps[:], gelu)
                    for j in range(KC):
                        jj = KC * g + j
                        nc.tensor.matmul(
                            osum[:],
                            w2t[:, HC * c + jj:HC * c + jj + 1],
                            ht[:, j * TW:(j + 1) * TW],
                            start=(jj == 0),
                            stop=(jj == HC - 1),
                        )
                off = (c * NTT + tt) * TW
                nc.vector.tensor_copy(oall[:, off:off + TW], osum[:])

        osplit = max(1, (CPC - 2)) * NTT * TW
        nc.gpsimd.dma_start(out=out_d[:, :osplit], in_=oall[:, :osplit])
        nc.gpsimd.dma_start(out=out_d[:, osplit:], in_=oall[:, osplit:])

    nc.finalize()
    return nc


def _get_compiled(TW, NTT, dtype_name):
    key = (TW, NTT, dtype_name)
    if key not in _COMPILED:
        _COMPILED[key] = _build(TW, NTT, dtype_name)
    return _COMPILED[key]


def kernel(z, company_id, W1, b1, W2, b2):
    from concourse.bass_utils import run_bass_kernel_spmd

    z = np.asarray(z, dtype=np.float32)
    cid = np.asarray(company_id).astype(np.int64).ravel()
    W1 = np.asarray(W1, dtype=np.float32)
    b1 = np.asarray(b1, dtype=np.float32)
    W2 = np.asarray(W2, dtype=np.float32)
    b2 = np.asarray(b2, dtype=np.float32)
    O = W2.shape[2]

    np_op = np.float16
    dtype_name = "float16"

    idx_by_company = [np.nonzero(cid == gc)[0] for gc in range(C)]
    max_cnt = max((len(ix) for ix in idx_by_company), default=1)
    max_cnt = max(max_cnt, 1)
    if max_cnt <= 128:
        NTT = 1
        TW = ((max_cnt + 15) // 16) * 16
    else:
        NTT = (max_cnt + 127) // 128
        TW = 128
    CAP = NTT * TW

    nc = _get_compiled(TW, NTT, dtype_name)

    SELW = KC * TW
    B1W = CPC * 2 * 128
    sel = np.repeat(np.eye(KC, dtype=np_op), TW, axis=1)  # [KC, KC*TW]

    in_maps = []
    for core in range(NCORES):
        # zt[p, c, tt, k, t] = z[token, 128k+p]  (partition-major)
        zt = np.zeros((128, CPC, NTT, KC, TW), dtype=np_op)
        for ci in range(CPC):
            gc = core * CPC + ci
            ix = idx_by_company[gc]
            if len(ix) == 0:
                continue
            zpad = np.zeros((CAP, D), dtype=np_op)
            zpad[: len(ix)] = z[ix].astype(np_op)
            zt[:, ci] = zpad.reshape(NTT, TW, KC, 128).transpose(3, 0, 2, 1)
        # w1[c, p, g, k, hh] = W1[gc, 128k+p, 512g+hh]
        w1 = (
            W1[core * CPC:(core + 1) * CPC]
            .reshape(CPC, KC, 128, 2, H // 2)
            .transpose(0, 2, 3, 1, 4)
            .astype(np_op)
        )
        # b1h[k, c, g, m] = b1[gc, 512g+128k+m]
        b1h = (
            b1[core * CPC:(core + 1) * CPC]
            .reshape(CPC, 2, KC, 128)
            .transpose(2, 0, 1, 3)
            .astype(np_op)
        )
        # w2h[p, HC*c + j] = W2[gc, 128j+p, 0]
        w2h = (
            W2[core * CPC:(core + 1) * CPC, :, 0]
            .reshape(CPC, HC, 128)
            .transpose(2, 0, 1)
            .reshape(128, CPC * HC)
            .astype(np_op)
        )
        cst = np.zeros((KC, SELW + B1W), dtype=np_op)
        cst[:, 0:SELW] = sel
        cst[:, SELW:SELW + B1W] = b1h.reshape(KC, B1W)
        in_maps.append(
            {
                "zt": np.ascontiguousarray(zt),
                "w1": np.ascontiguousarray(w1),
                "cst": np.ascontiguousarray(cst),
                "w2h": np.ascontiguousarray(w2h),
            }
        )

    res = run_bass_kernel_spmd(nc, in_maps, list(range(NCORES)))

    out = np.zeros((B, O), dtype=np.float32)
    for core in range(NCORES):
        core_out = res.results[core]["out"].reshape(CPC, NTT * TW)
        for ci in range(CPC):
            gc = core * CPC + ci
            ix = idx_by_company[gc]
            if len(ix) == 0:
                continue
            out[ix, 0] = core_out[ci, : len(ix)] + b2[gc, 0]
    return out



# revision 2
# speedup vs baseline: 1.0257x; 1.0257x over previous
"""Trainium2 Bass kernel for CompanySpecificHeads (MoE-style routed MLP heads), v2.

Semantics (matching the reference):
    out[b] = gelu(z[b] @ W1[cid[b]] + b1[cid[b]]) @ W2[cid[b]] + b2[cid[b]]

Expert-parallel across 8 NeuronCores, 8 companies per core. Key changes vs v1:

  * W1 is streamed as float8 E3M4 (4 mantissa bits) with a fixed power-of-2
    prescale folded out in the gelu activation's scale. This halves the
    dominant HBM traffic (8MB -> 4MB per core) and halves LDWEIGHTS time
    (FWL reads 4 fp8/cycle vs 2 fp16). End-to-end rel err ~1.3e-2 < 2e-2.
    The moving operand (tokens) stays fp16 - mixed-dtype matmul is allowed.
  * Exact per-slot token widths instead of a global padded capacity:
    companies are sorted by token count into 8 slots of 8 (one company per
    core per slot); slot width = max count in slot, padded to 4. All cores
    share the same widths (SPMD single program); padding waste ~4%.
  * Bias via a K=4 selector matmul (psum has_written must be set by the PE),
    pre-scaled by SCALE so gelu(psum/SCALE) is exact.
  * L2 (w2 dot) stays on the PE, software-pipelined one company behind L1
    so the PE never waits on the ACT engine's gelu.
  * DMA: w1 per company on the sync HWDGE ring (first company first);
    consts + tail z on the scalar ring; first-two-slot z on gpsimd after the
    warmup memset. Output staged in SBUF, one HWDGE store at the end.
"""

import numpy as np

B, C, D, H = 4096, 64, 512, 1024
NCORES = 8
CPC = C // NCORES
KC = D // 128      # contraction chunks of 128
HC = H // 128      # h chunks of 128
SCALE = 16.0       # W1 prescale before e3m4 quantization
WARMUP = 6         # PE warmup matmuls (HAM clock ramp)

_COMPILED = {}


def _build(widths):
    """Build the Bass/Tile program for per-slot token widths `widths`."""
    import concourse.bass as bass
    import concourse.bacc as bacc
    import concourse.mybir as mybir
    from concourse.tile import TileContext
    from contextlib import ExitStack

    f32 = mybir.dt.float32
    f16 = mybir.dt.float16
    f8e3 = mybir.dt.float8e3

    NSLOT = len(widths)
    Wmax = max(widths)
    SELW = KC * Wmax
    B1W = NSLOT * 2 * 128
    cum = np.concatenate([[0], np.cumsum(widths)])
    NTOT = int(cum[-1])

    gelu = mybir.ActivationFunctionType.Gelu

    nc = bacc.Bacc(None, target_bir_lowering=False)

    zt_d = nc.dram_tensor("zt", [128, KC * NTOT], f16, kind="ExternalInput")
    w1_d = nc.dram_tensor("w1", [NSLOT, 128, 2 * KC * (H // 2)], f8e3,
                          kind="ExternalInput")
    cst_d = nc.dram_tensor("cst", [KC, SELW + B1W], f16, kind="ExternalInput")
    w2_d = nc.dram_tensor("w2", [128, NSLOT * HC], f16, kind="ExternalInput")
    out_d = nc.dram_tensor("out", [1, NTOT], f32, kind="ExternalOutput")

    with TileContext(nc) as tc, ExitStack() as ctx:
        const = ctx.enter_context(tc.tile_pool(name="const", bufs=1))

        # Warmup scratch: memset is gpsimd's first instruction so the PE can
        # start ramping the HAM clock right after engine boot.
        wsc = const.tile([128, 256], f16)
        nc.gpsimd.memset(wsc[:], 0.0)

        # Consts on the scalar HWDGE ring (tiny, land first).
        ct = const.tile([KC, SELW + B1W], f16)
        nc.scalar.dma_start(out=ct[:], in_=cst_d[:])
        w2t = const.tile([128, NSLOT * HC], f16)
        nc.scalar.dma_start(out=w2t[:], in_=w2_d[:])

        # Routed tokens: first two slots on gpsimd (right after memset) so
        # company 0 can start as soon as its weights land; rest on scalar.
        zall = const.tile([128, KC * NTOT], f16)
        z01 = int(KC * cum[min(2, NSLOT)])
        nc.gpsimd.dma_start(out=zall[:, :z01], in_=zt_d[:, :z01])
        if z01 < KC * NTOT:
            nc.scalar.dma_start(out=zall[:, z01:], in_=zt_d[:, z01:])

        # Staged per-slot outputs; single store at the end.
        oall = const.tile([1, NTOT], f32)

        # Per-company weights on the sync HWDGE ring, one DMA per company,
        # first company first (4KB contiguous per partition, full-rate).
        w1p = ctx.enter_context(tc.tile_pool(name="w1p", bufs=1))
        w1ts = []
        for s in range(NSLOT):
            w1t = w1p.tile([128, 2, KC, H // 2], f8e3, name=f"w1_{s}")
            nc.sync.dma_start(out=w1t[:], in_=w1_d[s])
            w1ts.append(w1t)

        hp = ctx.enter_context(tc.tile_pool(name="hp", bufs=6))
        pp = ctx.enter_context(tc.tile_pool(name="pp", bufs=4, space="PSUM"))
        opp = ctx.enter_context(tc.tile_pool(name="opp", bufs=2, space="PSUM"))
        wps = ctx.enter_context(tc.tile_pool(name="wps", bufs=1, space="PSUM"))

        wp = wps.tile([128, 256], f32)
        for _ in range(WARMUP):
            nc.tensor.matmul(wp[:], wsc[:, :128], wsc[:], start=True, stop=True)

        sel = ct[:, 0:SELW].rearrange("p (j t) -> p j t", j=KC)
        b1t = ct[:, SELW:SELW + B1W].rearrange("p (s g m) -> p s g m",
                                               s=NSLOT, g=2)

        def do_l2(s, W, off, hts):
            osum = opp.tile([1, Wmax], f32)
            for g in range(2):
                for j in range(KC):
                    jj = KC * g + j
                    nc.tensor.matmul(
                        osum[:, :W],
                        w2t[:, HC * s + jj:HC * s + jj + 1],
                        hts[g][:, j * W:(j + 1) * W],
                        start=(jj == 0),
                        stop=(jj == HC - 1),
                    )
            nc.vector.tensor_copy(oall[:, off:off + W], osum[:, :W])

        prev = None
        for s in range(NSLOT):
            W = widths[s]
            off = int(cum[s])
            zc = zall[:, KC * off:KC * (off + W)].rearrange(
                "p (k t) -> p k t", k=KC)
            w1t = w1ts[s]
            hts = []
            for g in range(2):
                ps = pp.tile([128, KC * Wmax], f32)
                psb = ps[:, 0:KC * W].rearrange("p (j t) -> p j t", j=KC)
                # bias: ps[128?+m, (j,t)] = SCALE*b1[...,128j+m] via selector
                nc.tensor.matmul(psb, b1t[:, s, g, :], sel[:, :, 0:W],
                                 start=True, stop=False)
                for k in range(KC):
                    for j in range(KC):
                        nc.tensor.matmul(
                            ps[:, j * W:(j + 1) * W],
                            w1t[:, g, k, 128 * j:128 * (j + 1)],
                            zc[:, k, :],
                            start=False,
                            stop=(k == KC - 1),
                        )
                ht = hp.tile([128, KC * Wmax], f16)
                nc.scalar.activation(ht[:, 0:KC * W], ps[:, 0:KC * W], gelu,
                                     scale=1.0 / SCALE)
                hts.append(ht)
            if prev is not None:
                do_l2(*prev)
            prev = (s, W, off, hts)
        do_l2(*prev)

        nc.scalar.dma_start(out=out_d[:], in_=oall[:])

    nc.finalize()
    return nc


def _get_compiled(widths):
    key = tuple(widths)
    if key not in _COMPILED:
        _COMPILED[key] = _build(list(widths))
    return _COMPILED[key]


def kernel(z, company_id, W1, b1, W2, b2):
    import ml_dtypes
    from concourse.bass_utils import run_bass_kernel_spmd

    z = np.asarray(z, dtype=np.float32)
    cid = np.asarray(company_id).astype(np.int64).ravel()
    W1 = np.asarray(W1, dtype=np.float32)
    b1 = np.asarray(b1, dtype=np.float32)
    W2 = np.asarray(W2, dtype=np.float32)
    b2 = np.asarray(b2, dtype=np.float32)
    O = W2.shape[2]

    idx_by_company = [np.nonzero(cid == gc)[0] for gc in range(C)]

    # Segment any company with >128 tokens (rare) into <=128-token chunks.
    segs = []  # (gc, tok_start, seg_len)
    for gc in range(C):
        n = len(idx_by_company[gc])
        st = 0
        while st < n or (st == 0 and n == 0):
            ln = min(128, n - st)
            segs.append((gc, st, ln))
            st += max(ln, 1)
            if n == 0:
                break
    # pad to a multiple of NCORES with dummy zero-token segments
    while len(segs) % NCORES != 0:
        segs.append((0, 0, 0))

    # Sort descending; slot k gets segs[8k:8k+8] (one per core); shared width.
    segs.sort(key=lambda t: -t[2])
    NSLOT = len(segs) // NCORES
    widths = []
    for k in range(NSLOT):
        mx = max(t[2] for t in segs[k * NCORES:(k + 1) * NCORES])
        widths.append(max(4, ((mx + 3) // 4) * 4))
    cum = np.concatenate([[0], np.cumsum(widths)])
    NTOT = int(cum[-1])
    Wmax = max(widths)
    SELW = KC * Wmax
    B1W = NSLOT * 2 * 128

    nc = _get_compiled(widths)

    # selector: sel[k, j*Wmax + t] = 1 if j == k
    sel = np.zeros((KC, KC, Wmax), dtype=np.float16)
    for k in range(KC):
        sel[k, k, :] = 1.0
    sel = sel.reshape(KC, SELW)

    in_maps = []
    core_slots = []  # per core: list of (gc, tok_indices) per slot
    for core in range(NCORES):
        slots = [segs[k * NCORES + core] for k in range(NSLOT)]
        core_slots.append(slots)

        zt = np.zeros((128, KC * NTOT), dtype=np.float16)
        w1 = np.zeros((NSLOT, 128, 2 * KC * (H // 2)),
                      dtype=ml_dtypes.float8_e3m4)
        b1h = np.zeros((KC, NSLOT, 2, 128), dtype=np.float16)
        w2h = np.zeros((128, NSLOT * HC), dtype=np.float16)

        for s, (gc, st, ln) in enumerate(slots):
            W = widths[s]
            if ln > 0:
                ix = idx_by_company[gc][st:st + ln]
                # zt block: [128, KC, W]; zt[p, k, t] = z[tok, 128k+p]
                zb = np.zeros((KC, 128, W), dtype=np.float16)
                zb[:, :, :ln] = (
                    z[ix].reshape(ln, KC, 128).transpose(1, 2, 0)
                )
                zt[:, KC * cum[s]:KC * (cum[s] + W)] = (
                    zb.transpose(1, 0, 2).reshape(128, KC * W)
                )
            # w1[s][p][g*KC*512 + k*512 + hh] = SCALE*W1[gc, 128k+p, 512g+hh]
            w1[s] = (
                (W1[gc] * SCALE)
                .reshape(KC, 128, 2, H // 2)
                .transpose(1, 2, 0, 3)
                .reshape(128, 2 * KC * (H // 2))
                .astype(ml_dtypes.float8_e3m4)
            )
            # b1h[k, s, g, m] = SCALE*b1[gc, 512g+128k+m]
            b1h[:, s] = (
                (b1[gc] * SCALE).reshape(2, KC, 128).transpose(1, 0, 2)
            ).astype(np.float16)
            # w2h[p, HC*s + jj] = W2[gc, 128jj+p, 0]
            w2h[:, HC * s:HC * (s + 1)] = (
                W2[gc, :, 0].reshape(HC, 128).T.astype(np.float16)
            )

        cst = np.zeros((KC, SELW + B1W), dtype=np.float16)
        cst[:, :SELW] = sel
        cst[:, SELW:] = b1h.reshape(KC, B1W)
        in_maps.append({
            "zt": np.ascontiguousarray(zt),
            "w1": np.ascontiguousarray(w1),
            "cst": np.ascontiguousarray(cst),
            "w2": np.ascontiguousarray(w2h),
        })

    res = run_bass_kernel_spmd(nc, in_maps, list(range(NCORES)))

    out = np.zeros((B, O), dtype=np.float32)
    for core in range(NCORES):
        core_out = res.results[core]["out"].reshape(-1)
        for s, (gc, st, ln) in enumerate(core_slots[core]):
            if ln == 0:
                continue
            ix = idx_by_company[gc][st:st + ln]
            out[ix, 0] = core_out[cum[s]:cum[s] + ln] + b2[gc, 0]
    return out


# revision 9
# speedup vs baseline: 1.2771x; 1.2451x over previous
"""Trainium2 Bass kernel for CompanySpecificHeads (MoE-style routed MLP heads), v2.

Semantics (matching the reference):
    out[b] = gelu(z[b] @ W1[cid[b]] + b1[cid[b]]) @ W2[cid[b]] + b2[cid[b]]

Expert-parallel across 8 NeuronCores, 8 companies per core. Key changes vs v1:

  * W1 is streamed as float8 E3M4 (4 mantissa bits) with a fixed power-of-2
    prescale folded out in the gelu activation's scale. This halves the
    dominant HBM traffic (8MB -> 4MB per core) and halves LDWEIGHTS time
    (FWL reads 4 fp8/cycle vs 2 fp16). End-to-end rel err ~1.3e-2 < 2e-2.
    The moving operand (tokens) stays fp16 - mixed-dtype matmul is allowed.
  * Exact per-slot token widths instead of a global padded capacity:
    companies are sorted by token count into 8 slots of 8 (one company per
    core per slot); slot width = max count in slot, padded to 4. All cores
    share the same widths (SPMD single program); padding waste ~4%.
  * Bias via a K=4 selector matmul (psum has_written must be set by the PE),
    pre-scaled by SCALE so gelu(psum/SCALE) is exact.
  * L2 (w2 dot) stays on the PE, software-pipelined one company behind L1
    so the PE never waits on the ACT engine's gelu.
  * DMA: w1 per company on the sync HWDGE ring (first company first);
    consts + tail z on the scalar ring; first-two-slot z on gpsimd after the
    warmup memset. Output staged in SBUF, one HWDGE store at the end.
"""

import numpy as np

B, C, D, H = 4096, 64, 512, 1024
NCORES = 8
CPC = C // NCORES
KC = D // 128      # contraction chunks of 128
HC = H // 128      # h chunks of 128
SCALE = 16.0       # W1 prescale before e3m4 quantization
# PE warmup: HAM un-throttles (1.2->2.4GHz) only after ~3.4us of sustained
# fp16-path matmul activity, and the e3m4 L1 matmuls do NOT register as
# activity (measured: 27us of dense e3m4 matmuls never flipped the clock).
# So burn a full window with dense fp16 warmup matmuls; the fp16 bias and
# L2 matmuls every ~1us keep it warm afterwards.
WARMUP = 12
WARMW = 512

_COMPILED = {}


def _build(widths):
    """Build the Bass/Tile program for per-slot token widths `widths`."""
    import concourse.bass as bass
    import concourse.bacc as bacc
    import concourse.mybir as mybir
    from concourse.tile import TileContext
    from contextlib import ExitStack

    f32 = mybir.dt.float32
    f16 = mybir.dt.float16
    f8e3 = mybir.dt.float8e3

    NSLOT = len(widths)
    Wmax = max(widths)
    SELW = KC * Wmax
    B1W = NSLOT * 2 * 128
    cum = np.concatenate([[0], np.cumsum(widths)])
    NTOT = int(cum[-1])

    gelu = mybir.ActivationFunctionType.Gelu

    nc = bacc.Bacc(None, target_bir_lowering=False)

    zt_d = nc.dram_tensor("zt", [128, KC * NTOT], f16, kind="ExternalInput")
    w1_d = nc.dram_tensor("w1", [NSLOT, 128, 2 * KC * (H // 2)], f8e3,
                          kind="ExternalInput")
    cst_d = nc.dram_tensor("cst", [KC, SELW + B1W], f16, kind="ExternalInput")
    w2_d = nc.dram_tensor("w2", [128, NSLOT * HC], f16, kind="ExternalInput")
    out_d = nc.dram_tensor("out", [1, NTOT], f32, kind="ExternalOutput")

    with TileContext(nc) as tc, ExitStack() as ctx:
        const = ctx.enter_context(tc.tile_pool(name="const", bufs=1))

        # Warmup scratch: memset is gpsimd's first instruction so the PE can
        # start ramping the HAM clock right after engine boot.
        wsc = const.tile([128, WARMW], f16)
        nc.gpsimd.memset(wsc[:], 0.0)

        # Scalar HWDGE ring, in consumption order: first two slots of z,
        # then the small consts, then the remaining z.
        zall = const.tile([128, KC * NTOT], f16)
        z01 = int(KC * cum[min(2, NSLOT)])
        nc.scalar.dma_start(out=zall[:, :z01], in_=zt_d[:, :z01])
        ct = const.tile([KC, SELW + B1W], f16)
        nc.scalar.dma_start(out=ct[:], in_=cst_d[:])
        w2t = const.tile([128, NSLOT * HC], f16)
        nc.scalar.dma_start(out=w2t[:], in_=w2_d[:])
        if z01 < KC * NTOT:
            nc.scalar.dma_start(out=zall[:, z01:], in_=zt_d[:, z01:])

        # Staged per-slot outputs; single store at the end.
        oall = const.tile([1, NTOT], f32)

        # Per-company weights on the sync HWDGE ring, one DMA per company,
        # first company first (4KB contiguous per partition, full-rate).
        w1p = ctx.enter_context(tc.tile_pool(name="w1p", bufs=1))
        w1ts = []
        for s in range(NSLOT):
            w1t = w1p.tile([128, 2, KC, H // 2], f8e3, name=f"w1_{s}")
            nc.sync.dma_start(out=w1t[:], in_=w1_d[s])
            w1ts.append(w1t)

        hp = ctx.enter_context(tc.tile_pool(name="hp", bufs=6))
        pp = ctx.enter_context(tc.tile_pool(name="pp", bufs=4, space="PSUM"))
        opp = ctx.enter_context(tc.tile_pool(name="opp", bufs=2, space="PSUM"))
        wps = ctx.enter_context(tc.tile_pool(name="wps", bufs=1, space="PSUM"))

        wp = wps.tile([128, WARMW], f32)
        for _ in range(WARMUP):
            nc.tensor.matmul(wp[:], wsc[:, :128], wsc[:], start=True, stop=True)

        sel = ct[:, 0:SELW].rearrange("p (j t) -> p j t", j=KC)
        b1t = ct[:, SELW:SELW + B1W].rearrange("p (s g m) -> p s g m",
                                               s=NSLOT, g=2)

        def do_l2(s, W, off, hts):
            osum = opp.tile([1, Wmax], f32)
            for g in range(2):
                for j in range(KC):
                    jj = KC * g + j
                    nc.tensor.matmul(
                        osum[:, :W],
                        w2t[:, HC * s + jj:HC * s + jj + 1],
                        hts[g][:, j * W:(j + 1) * W],
                        start=(jj == 0),
                        stop=(jj == HC - 1),
                    )
            nc.vector.tensor_copy(oall[:, off:off + W], osum[:, :W])

        stored = [False]
        prev = None
        for s in range(NSLOT):
            W = widths[s]
            off = int(cum[s])
            zc = zall[:, KC * off:KC * (off + W)].rearrange(
                "p (k t) -> p k t", k=KC)
            w1t = w1ts[s]
            hts = []
            for g in range(2):
                ps = pp.tile([128, KC * Wmax], f32)
                psb = ps[:, 0:KC * W].rearrange("p (j t) -> p j t", j=KC)
                # bias: ps[128?+m, (j,t)] = SCALE*b1[...,128j+m] via selector
                nc.tensor.matmul(psb, b1t[:, s, g, :], sel[:, :, 0:W],
                                 start=True, stop=False)
                for k in range(KC):
                    for j in range(KC):
                        nc.tensor.matmul(
                            ps[:, j * W:(j + 1) * W],
                            w1t[:, g, k, 128 * j:128 * (j + 1)],
                            zc[:, k, :],
                            start=False,
                            stop=(k == KC - 1),
                        )
                ht = hp.tile([128, KC * Wmax], f16)
                nc.scalar.activation(ht[:, 0:KC * W], ps[:, 0:KC * W], gelu,
                                     scale=1.0 / SCALE)
                hts.append(ht)
            if prev is not None:
                do_l2(*prev)
                # early partial store: overlap the store's issue+receipt
                # latency with the last companies' compute
                if prev[0] == NSLOT - 3:
                    so = int(cum[NSLOT - 2])
                    nc.scalar.dma_start(out=out_d[:, :so], in_=oall[:, :so])
                    stored[0] = so
            prev = (s, W, off, hts)
        do_l2(*prev)

        so = stored[0] or 0
        nc.scalar.dma_start(out=out_d[:, so:], in_=oall[:, so:])

    nc.finalize()
    return nc


def _get_compiled(widths):
    key = tuple(widths)
    if key not in _COMPILED:
        _COMPILED[key] = _build(list(widths))
    return _COMPILED[key]


def kernel(z, company_id, W1, b1, W2, b2):
    import ml_dtypes
    from concourse.bass_utils import run_bass_kernel_spmd

    z = np.asarray(z, dtype=np.float32)
    cid = np.asarray(company_id).astype(np.int64).ravel()
    W1 = np.asarray(W1, dtype=np.float32)
    b1 = np.asarray(b1, dtype=np.float32)
    W2 = np.asarray(W2, dtype=np.float32)
    b2 = np.asarray(b2, dtype=np.float32)
    O = W2.shape[2]

    idx_by_company = [np.nonzero(cid == gc)[0] for gc in range(C)]

    # Segment any company with >128 tokens (rare) into <=128-token chunks.
    segs = []  # (gc, tok_start, seg_len)
    for gc in range(C):
        n = len(idx_by_company[gc])
        st = 0
        while st < n or (st == 0 and n == 0):
            ln = min(128, n - st)
            segs.append((gc, st, ln))
            st += max(ln, 1)
            if n == 0:
                break
    # pad to a multiple of NCORES with dummy zero-token segments
    while len(segs) % NCORES != 0:
        segs.append((0, 0, 0))

    # Sort descending; slot k gets segs[8k:8k+8] (one per core); shared width.
    segs.sort(key=lambda t: -t[2])
    NSLOT = len(segs) // NCORES
    widths = []
    for k in range(NSLOT):
        mx = max(t[2] for t in segs[k * NCORES:(k + 1) * NCORES])
        widths.append(max(4, ((mx + 3) // 4) * 4))
    cum = np.concatenate([[0], np.cumsum(widths)])
    NTOT = int(cum[-1])
    Wmax = max(widths)
    SELW = KC * Wmax
    B1W = NSLOT * 2 * 128

    nc = _get_compiled(widths)

    # selector: sel[k, j*Wmax + t] = 1 if j == k
    sel = np.zeros((KC, KC, Wmax), dtype=np.float16)
    for k in range(KC):
        sel[k, k, :] = 1.0
    sel = sel.reshape(KC, SELW)

    in_maps = []
    core_slots = []  # per core: list of (gc, tok_indices) per slot
    for core in range(NCORES):
        slots = [segs[k * NCORES + core] for k in range(NSLOT)]
        core_slots.append(slots)

        zt = np.zeros((128, KC * NTOT), dtype=np.float16)
        w1 = np.zeros((NSLOT, 128, 2 * KC * (H // 2)),
                      dtype=ml_dtypes.float8_e3m4)
        b1h = np.zeros((KC, NSLOT, 2, 128), dtype=np.float16)
        w2h = np.zeros((128, NSLOT * HC), dtype=np.float16)

        for s, (gc, st, ln) in enumerate(slots):
            W = widths[s]
            if ln > 0:
                ix = idx_by_company[gc][st:st + ln]
                # zt block: [128, KC, W]; zt[p, k, t] = z[tok, 128k+p]
                zb = np.zeros((KC, 128, W), dtype=np.float16)
                zb[:, :, :ln] = (
                    z[ix].reshape(ln, KC, 128).transpose(1, 2, 0)
                )
                zt[:, KC * cum[s]:KC * (cum[s] + W)] = (
                    zb.transpose(1, 0, 2).reshape(128, KC * W)
                )
            # w1[s][p][g*KC*512 + k*512 + hh] = SCALE*W1[gc, 128k+p, 512g+hh]
            w1[s] = (
                (W1[gc] * SCALE)
                .reshape(KC, 128, 2, H // 2)
                .transpose(1, 2, 0, 3)
                .reshape(128, 2 * KC * (H // 2))
                .astype(ml_dtypes.float8_e3m4)
            )
            # b1h[k, s, g, m] = SCALE*b1[gc, 512g+128k+m]
            b1h[:, s] = (
                (b1[gc] * SCALE).reshape(2, KC, 128).transpose(1, 0, 2)
            ).astype(np.float16)
            # w2h[p, HC*s + jj] = W2[gc, 128jj+p, 0]
            w2h[:, HC * s:HC * (s + 1)] = (
                W2[gc, :, 0].reshape(HC, 128).T.astype(np.float16)
            )

        cst = np.zeros((KC, SELW + B1W), dtype=np.float16)
        cst[:, :SELW] = sel
        cst[:, SELW:] = b1h.reshape(KC, B1W)
        in_maps.append({
            "zt": np.ascontiguousarray(zt),
            "w1": np.ascontiguousarray(w1),
            "cst": np.ascontiguousarray(cst),
            "w2": np.ascontiguousarray(w2h),
        })

    res = run_bass_kernel_spmd(nc, in_maps, list(range(NCORES)))

    out = np.zeros((B, O), dtype=np.float32)
    for core in range(NCORES):
        core_out = res.results[core]["out"].reshape(-1)
        for s, (gc, st, ln) in enumerate(core_slots[core]):
            if ln == 0:
                continue
            ix = idx_by_company[gc][st:st + ln]
            out[ix, 0] = core_out[cum[s]:cum[s] + ln] + b2[gc, 0]
    return out
